# revision 15
# baseline (speedup 1.0000x reference)
"""Trainium2 Bass kernel for nn_AbPred_both (dense per-branch MLP + LayerNorm + row gather).

Strategy:
  - Only the P=2048 rows selected by positions_y are computed (the reference's
    computation is row-wise, so Y[positions_y] == f(X[positions_y], X1[positions_y])).
  - Branch (antibody) axis A=16 is sharded across 8 cores, 2 branches per core.
  - All activations live transposed on-chip ([H, batch]) so every matmul
    contracts over the partition axis, with the stored weight layout [H_in, H_out]
    serving directly as lhsT.
  - Matmuls run in float32r (full PE rate, ~tf32 accuracy; measured 1.5e-4
    per-layer rel err on HW vs 2.4e-3 for bf16).
  - LayerNorm is folded into the final tiny [H,4] matmul:
        y^T = rstd * (W4g^T h^T) - (rstd*mu) * c1 + c2
    with W4g = W4*gamma, c1 = sum_h W4g, c2 = W4^T beta + b4 (host precomputed),
    and mu/rstd from column sums of h and h*h obtained via a ones column packed
    into the W4g matmul (at partition 32 so PSUM reads stay 32-aligned).
"""
import numpy as np
import ml_dtypes

bf16 = ml_dtypes.bfloat16

H = 1024
A = 16
O = 4
B = 4096
P = 2048
EPS = 1e-5
NCORES = 8
BR = A // NCORES     # branches per core
NB = 512             # batch chunk (columns per matmul)
NCH = P // NB        # chunks
KT = H // 128
MT = H // 128

MODE = "f32r"        # one of: bf16, split3, f32r, f32

_cache = {}
_last_in_maps = None


def _hi(a):
    return np.ascontiguousarray(a.astype(bf16))


def _lo(a):
    return np.ascontiguousarray(
        (a.astype(np.float64) - a.astype(bf16).astype(np.float64))
        .astype(np.float32).astype(bf16))


def _build(mode, reps=1):
    import concourse.mybir as mybir
    from concourse import bacc
    from concourse.alu_op_type import AluOpType
    from concourse.tile import TileContext

    F32 = mybir.dt.float32
    F32R = mybir.dt.float32r
    BF = mybir.dt.bfloat16
    AF = mybir.ActivationFunctionType

    lowbits = mode == "split3"
    if mode == "f32r":
        xdt = wdt = F32R
    elif mode == "f32":
        xdt = wdt = F32
    else:
        xdt = wdt = BF

    nc = bacc.Bacc("TRN2", target_bir_lowering=False, debug=False,
                   num_devices=NCORES)

    def din(name, shape, dt):
        return nc.dram_tensor(name, shape, dt, kind="ExternalInput").ap()

    d_x = {}
    for nm in ("Xh", "X1h"):
        d_x[nm] = din(nm, [H, P], xdt)
    if lowbits:
        for nm in ("Xl", "X1l"):
            d_x[nm] = din(nm, [H, P], xdt)
    d_w = {}
    for nm in ("W1h", "W2h", "W3h"):
        d_w[nm] = din(nm, [BR, H, H], wdt)
    if lowbits:
        for nm in ("W1l", "W2l", "W3l"):
            d_w[nm] = din(nm, [BR, H, H], wdt)
    d_bc12 = din("bc12", [BR, 128, MT], F32)
    d_bc3 = din("bc3", [BR, 128, MT], F32)
    d_w45h = din("W45h", [BR, H, 33], xdt)
    d_w45l = din("W45l", [BR, H, 33], xdt) if lowbits else None
    d_c12 = din("c12", [BR, 4, 2], F32)
    d_ones = din("onesd", [128, 1], xdt)
    d_y = nc.dram_tensor("y", [4, BR * P], F32, kind="ExternalOutput").ap()
    d_bounce = nc.dram_tensor("bounce", [BR * NCH, 2 * NB], F32).ap()

    with TileContext(nc) as tc:
        wb = 2 if mode == "bf16" else 1
        xb = 2 if mode == "bf16" else 1
        with (
            tc.tile_pool(name="wpool", bufs=wb) as wpool,
            tc.tile_pool(name="xpool", bufs=xb) as xpool,
            tc.tile_pool(name="zpool", bufs=1) as zpool,
            tc.tile_pool(name="tpool", bufs=2) as tpool,
            tc.tile_pool(name="psum", bufs=4, space="PSUM") as pp,
            tc.tile_pool(name="psq", bufs=2, space="PSUM") as pps,
            tc.tile_pool(name="small", bufs=1) as sp,
            tc.tile_pool(name="persist", bufs=1) as pers,
        ):
            # persistent small tensors
            bc12 = pers.tile([128, BR * MT], F32, name="bc12s")
            bc3 = pers.tile([128, BR * MT], F32, name="bc3s")
            c12 = pers.tile([4, BR * 2], F32, name="c12s")
            for s in range(BR):
                nc.gpsimd.dma_start(out=bc12[:, s * MT:(s + 1) * MT], in_=d_bc12[s])
                nc.gpsimd.dma_start(out=bc3[:, s * MT:(s + 1) * MT], in_=d_bc3[s])
                nc.gpsimd.dma_start(out=c12[:, s * 2:(s + 1) * 2], in_=d_c12[s])
            w45h = [pers.tile([128, BR * 33], xdt, name=f"w45h{k}")
                    for k in range(KT)]
            w45l = ([pers.tile([128, BR * 33], xdt, name=f"w45l{k}")
                     for k in range(KT)] if lowbits else None)
            for k in range(KT):
                for s in range(BR):
                    nc.gpsimd.dma_start(out=w45h[k][:, s * 33:(s + 1) * 33],
                                        in_=d_w45h[s, k * 128:(k + 1) * 128, :])
                    if lowbits:
                        nc.gpsimd.dma_start(out=w45l[k][:, s * 33:(s + 1) * 33],
                                            in_=d_w45l[s, k * 128:(k + 1) * 128, :])
            ones1 = pers.tile([128, 1], xdt, name="ones1")
            nc.gpsimd.dma_start(out=ones1[:], in_=d_ones[:])
            epst = pers.tile([1, 1], F32, name="epst")
            nc.vector.memset(epst[:], EPS)
            y_sb = pers.tile([4, BR * P], F32, name="y_sb")

            env = {
                "xdt": xdt, "wdt": wdt, "d_x": d_x, "d_w": d_w,
                "d_bounce": d_bounce, "bc12": bc12, "bc3": bc3, "c12": c12,
                "w45h": w45h, "w45l": w45l, "ones1": ones1, "epst": epst,
                "y_sb": y_sb, "wpool": wpool, "xpool": xpool, "zpool": zpool,
                "tpool": tpool, "pp": pp, "pps": pps, "sp": sp,
            }
            for rep in range(reps):
                for s in range(BR):
                    _branch(nc, mode, s, env)

            nc.sync.dma_start(out=d_y[:], in_=y_sb[:])

    nc.compile()
    return nc


def _branch(nc, mode, s, env):
    """One branch (2 per core): z -> h -> LN-folded y, streamed over batch chunks."""
    import concourse.mybir as mybir
    from concourse.alu_op_type import AluOpType

    F32 = mybir.dt.float32
    AF = mybir.ActivationFunctionType
    lowbits = mode == "split3"
    xdt = env["xdt"]
    wdt = env["wdt"]
    wpool, xpool, zpool, tpool = (env["wpool"], env["xpool"], env["zpool"],
                                  env["tpool"])
    pp, pps, sp = env["pp"], env["pps"], env["sp"]
    d_x, d_w, d_bounce = env["d_x"], env["d_w"], env["d_bounce"]
    bc12, bc3, c12 = env["bc12"], env["bc3"], env["c12"]
    w45h, w45l, ones1, epst, y_sb = (env["w45h"], env["w45l"], env["ones1"],
                                     env["epst"], env["y_sb"])

    def load_x(ch):
        xt = {}
        for nm in d_x:
            xt[nm] = [xpool.tile([128, NB], xdt, name=f"{nm}c{k}", tag=f"{nm}c{k}")
                      for k in range(KT)]
            for k in range(KT):
                nc.sync.dma_start(
                    out=xt[nm][k][:],
                    in_=d_x[nm][k * 128:(k + 1) * 128, ch * NB:(ch + 1) * NB])
        return xt

    # chunk-0 X is issued BEFORE the weights: the first z matmuls need it and
    # it is much smaller than the weight set. W3 loads last (h-phase only).
    xt0 = load_x(0)
    wt = {}
    wnames = (("W1h", "W2h", "W1l", "W2l", "W3h", "W3l")
              if lowbits else ("W1h", "W2h", "W3h"))
    for nm in wnames:
        wt[nm] = [wpool.tile([128, H], wdt, name=f"{nm}k{k}", tag=f"{nm}k{k}")
                  for k in range(KT)]
        for k in range(KT):
            nc.sync.dma_start(out=wt[nm][k][:],
                              in_=d_w[nm][s, k * 128:(k + 1) * 128, :])

    for ch in range(NCH):
        cb = s * NCH + ch
        xt = xt0 if ch == 0 else load_x(ch)

        if lowbits:
            zpasses = [("W1h", "Xh"), ("W1l", "Xh"), ("W1h", "Xl"),
                       ("W2h", "X1h"), ("W2l", "X1h"), ("W2h", "X1l")]
        else:
            zpasses = [("W1h", "Xh"), ("W2h", "X1h")]

        # ---- z = W1^T X + W2^T X1 (+bias) ----
        zh = []
        zl = []
        for m in range(MT):
            ps = pp.tile([128, NB], F32, name="ps", tag="ps")
            npass = len(zpasses)
            for pi, (wn, xn) in enumerate(zpasses):
                for k in range(KT):
                    nc.tensor.matmul(
                        ps[:],
                        lhsT=wt[wn][k][:, m * 128:(m + 1) * 128],
                        rhs=xt[xn][k][:],
                        start=(pi == 0 and k == 0),
                        stop=(pi == npass - 1 and k == KT - 1))
            zh_m = zpool.tile([128, NB], xdt, name=f"zh{m}", tag=f"zh{m}")
            bcol = bc12[:, s * MT + m:s * MT + m + 1]
            nc.scalar.activation(out=zh_m[:], in_=ps[:],
                                 func=AF.Identity, bias=bcol, scale=1.0)
            zh.append(zh_m)
            if lowbits:
                zl_m = zpool.tile([128, NB], xdt, name=f"zl{m}", tag=f"zl{m}")
                nc.vector.scalar_tensor_tensor(
                    out=zl_m[:], in0=ps[:], scalar=bcol, in1=zh_m[:],
                    op0=AluOpType.add, op1=AluOpType.subtract)
                zl.append(zl_m)

        # ---- h = relu(W3^T z + b3) ----
        hh = []
        hl = []
        ps_q = pps.tile([1, NB], F32, name="ps_q", tag="ps_q")
        for m in range(MT):
            ps = pp.tile([128, NB], F32, name="ps", tag="ps")
            hsrc = ([("W3h", zh), ("W3l", zh), ("W3h", zl)]
                    if lowbits else [("W3h", zh)])
            npass = len(hsrc)
            for pi, (wn, zsrc) in enumerate(hsrc):
                for k in range(KT):
                    nc.tensor.matmul(
                        ps[:],
                        lhsT=wt[wn][k][:, m * 128:(m + 1) * 128],
                        rhs=zsrc[k][:],
                        start=(pi == 0 and k == 0),
                        stop=(pi == npass - 1 and k == KT - 1))
            bcol = bc3[:, s * MT + m:s * MT + m + 1]
            hh_m = zpool.tile([128, NB], xdt, name=f"hh{m}", tag=f"hh{m}")
            if lowbits:
                hf_m = tpool.tile([128, NB], F32, name="hf", tag="hf")
                nc.vector.tensor_scalar(out=hf_m[:], in0=ps[:],
                                        scalar1=bcol, scalar2=0.0,
                                        op0=AluOpType.add, op1=AluOpType.max)
                nc.scalar.activation(out=hh_m[:], in_=hf_m[:],
                                     func=AF.Identity, bias=0.0, scale=1.0)
                hl_m = zpool.tile([128, NB], xdt, name=f"hl{m}", tag=f"hl{m}")
                nc.vector.tensor_tensor(hl_m[:], hf_m[:], hh_m[:],
                                        AluOpType.subtract)
                hl.append(hl_m)
            else:
                nc.scalar.activation(out=hh_m[:], in_=ps[:],
                                     func=AF.Relu, bias=bcol, scale=1.0)
            hh.append(hh_m)
            vv_m = tpool.tile([128, NB], xdt, name="vv", tag="vv")
            nc.vector.tensor_mul(vv_m[:], hh_m[:], hh_m[:])
            nc.tensor.matmul(ps_q[:], lhsT=ones1[:], rhs=vv_m[:],
                             start=(m == 0), stop=(m == MT - 1))

        # ---- u = [W4g|1]^T h (ones at partition 32) ; q = 1^T (h*h) ----
        ps_u = pps.tile([33, NB], F32, name="ps_u", tag="ps_u")
        usrc = ([(w45h, hh), (w45l, hh), (w45h, hl)]
                if lowbits else [(w45h, hh)])
        npass = len(usrc)
        for pi, (wsrc, hsrc) in enumerate(usrc):
            for k in range(KT):
                nc.tensor.matmul(
                    ps_u[:],
                    lhsT=wsrc[k][:, s * 33:(s + 1) * 33],
                    rhs=hsrc[k][:],
                    start=(pi == 0 and k == 0),
                    stop=(pi == npass - 1 and k == KT - 1))

        # ---- LN tail: y = rstd*u - (rstd*mu)*c1 + c2 ----
        mu = sp.tile([1, NB], F32, name="mu", tag="mu")
        tt = sp.tile([1, NB], F32, name="tt", tag="tt")
        row2 = sp.tile([1, 2 * NB], F32, name="row2", tag="row2")
        nc.scalar.mul(mu[:], ps_u[32:33, :], 1.0 / H)
        nc.scalar.mul(tt[:], ps_q[:], 1.0 / H)
        nc.vector.tensor_mul(row2[:, 0:NB], mu[:], mu[:])
        nc.vector.tensor_tensor(tt[:], tt[:], row2[:, 0:NB], AluOpType.subtract)
        nc.scalar.activation(out=tt[:], in_=tt[:], func=AF.Sqrt,
                             bias=epst[0:1, :], scale=1.0)
        nc.vector.reciprocal(out=row2[:, 0:NB], in_=tt[:])
        nc.vector.tensor_mul(row2[:, NB:2 * NB], mu[:], row2[:, 0:NB])
        bc4 = sp.tile([4, 2 * NB], F32, name="bc4", tag="bc4")
        nc.gpsimd.dma_start(out=d_bounce[cb:cb + 1, :], in_=row2[:])
        nc.gpsimd.dma_start(out=bc4[:],
                            in_=d_bounce[cb:cb + 1, :].partition_broadcast(4))
        t0 = sp.tile([4, NB], F32, name="t0", tag="t0")
        nc.vector.tensor_mul(t0[:], ps_u[0:4, :], bc4[:, 0:NB])
        nc.vector.scalar_tensor_tensor(
            out=t0[:], in0=bc4[:, NB:2 * NB],
            scalar=c12[:, s * 2:s * 2 + 1], in1=t0[:],
            op0=AluOpType.mult, op1=AluOpType.subtract)
        nc.scalar.activation(
            out=y_sb[:, s * P + ch * NB:s * P + (ch + 1) * NB],
            in_=t0[:], func=AF.Identity,
            bias=c12[:, s * 2 + 1:s * 2 + 2], scale=-1.0)


def kernel(X, X1, W1, b1, W2, b2, W3, b3, gamma, beta, W4, b4, positions_y):
    from concourse.bass_utils import run_bass_kernel_spmd

    mode = MODE
    lowbits = mode == "split3"
    fp32in = mode in ("f32r", "f32")

    if mode not in _cache:
        _cache[mode] = _build(mode)
    nc = _cache[mode]

    pos = np.asarray(positions_y).astype(np.int64)
    Xt = np.ascontiguousarray(np.asarray(X, np.float32)[pos].T)     # [H, P]
    X1t = np.ascontiguousarray(np.asarray(X1, np.float32)[pos].T)

    def cvt(a):
        return np.ascontiguousarray(a, np.float32) if fp32in else _hi(a)

    in_maps = []
    for c in range(NCORES):
        m = {"Xh": cvt(Xt), "X1h": cvt(X1t)}
        if lowbits:
            m["Xl"] = _lo(Xt)
            m["X1l"] = _lo(X1t)
        for base, Wt in (("W1", W1), ("W2", W2), ("W3", W3)):
            sl = np.asarray(Wt, np.float32)[2 * c:2 * c + 2]        # [BR, H, H]
            m[base + "h"] = cvt(sl)
            if lowbits:
                m[base + "l"] = _lo(sl)
        b12 = np.asarray(b1, np.float32)[2 * c:2 * c + 2] + \
            np.asarray(b2, np.float32)[2 * c:2 * c + 2]             # [BR, H]
        m["bc12"] = np.ascontiguousarray(
            b12.reshape(BR, MT, 128).transpose(0, 2, 1))            # [BR,128,MT]
        b3s = np.asarray(b3, np.float32)[2 * c:2 * c + 2]
        m["bc3"] = np.ascontiguousarray(
            b3s.reshape(BR, MT, 128).transpose(0, 2, 1))
        w45h = np.zeros((BR, H, 33), np.float32)
        c12a = np.zeros((BR, 4, 2), np.float32)
        for s in range(BR):
            a = 2 * c + s
            W4g = (np.asarray(W4, np.float32)[a]
                   * np.asarray(gamma, np.float32)[a][:, None])     # [H, 4]
            w45h[s, :, 0:4] = W4g
            w45h[s, :, 32] = 1.0
            c12a[s, :, 0] = W4g.sum(0)
            c12a[s, :, 1] = (np.asarray(W4, np.float32)[a].T
                             @ np.asarray(beta, np.float32)[a]) \
                + np.asarray(b4, np.float32)[a]
        if fp32in:
            m["W45h"] = np.ascontiguousarray(w45h)
        else:
            m["W45h"] = _hi(w45h)
        if lowbits:
            wl = _lo(w45h)
            wl[:, :, 32] = 0.0   # ones column must not double-count
            m["W45l"] = wl
        m["c12"] = np.ascontiguousarray(c12a)
        m["onesd"] = (np.ones((128, 1), np.float32) if fp32in
                      else np.ones((128, 1), bf16))
        in_maps.append(m)

    global _last_in_maps
    _last_in_maps = in_maps
    res = run_bass_kernel_spmd(nc, in_maps, list(range(NCORES)))

    out = np.empty((P, A * O), np.float32)
    for c in range(NCORES):
        yc = res.results[c]["y"]                                    # [4, BR*P]
        for s in range(BR):
            a = 2 * c + s
            out[:, a * O:(a + 1) * O] = yc[:, s * P:(s + 1) * P].T
    return out


# revision 17
# speedup vs baseline: 1.0403x; 1.0403x over previous
"""Trainium2 Bass kernel for nn_AbPred_both (dense per-branch MLP + LayerNorm + row gather).

Strategy:
  - Only the P=2048 rows selected by positions_y are computed (the reference's
    computation is row-wise, so Y[positions_y] == f(X[positions_y], X1[positions_y])).
  - Branch (antibody) axis A=16 is sharded across 8 cores, 2 branches per core.
  - All activations live transposed on-chip ([H, batch]) so every matmul
    contracts over the partition axis, with the stored weight layout [H_in, H_out]
    serving directly as lhsT.
  - Matmuls run in float32r (full PE rate, ~tf32 accuracy; measured 1.5e-4
    per-layer rel err on HW vs 2.4e-3 for bf16).
  - LayerNorm is folded into the final tiny [H,4] matmul:
        y^T = rstd * (W4g^T h^T) - (rstd*mu) * c1 + c2
    with W4g = W4*gamma, c1 = sum_h W4g, c2 = W4^T beta + b4 (host precomputed),
    and mu/rstd from column sums of h and h*h obtained via a ones column packed
    into the W4g matmul (at partition 32 so PSUM reads stay 32-aligned).
"""
import numpy as np
import ml_dtypes

bf16 = ml_dtypes.bfloat16

H = 1024
A = 16
O = 4
B = 4096
P = 2048
EPS = 1e-5
NCORES = 8
BR = A // NCORES     # branches per core
NB = 512             # batch chunk (columns per matmul)
NCH = P // NB        # chunks
KT = H // 128
MT = H // 128

MODE = "f32r"        # one of: bf16, split3, f32r, f32

_cache = {}
_last_in_maps = None
_last_nc = None
_last_plan = None


def _hi(a):
    return np.ascontiguousarray(a.astype(bf16))


def _lo(a):
    return np.ascontiguousarray(
        (a.astype(np.float64) - a.astype(bf16).astype(np.float64))
        .astype(np.float32).astype(bf16))


def _build(mode, reps=1, plan=(512, 512, 512, 512)):
    CAP = sum(plan)
    import concourse.mybir as mybir
    from concourse import bacc
    from concourse.alu_op_type import AluOpType
    from concourse.tile import TileContext

    F32 = mybir.dt.float32
    F32R = mybir.dt.float32r
    BF = mybir.dt.bfloat16
    AF = mybir.ActivationFunctionType

    lowbits = mode == "split3"
    if mode == "f32r":
        xdt = wdt = F32R
    elif mode == "f32":
        xdt = wdt = F32
    else:
        xdt = wdt = BF

    nc = bacc.Bacc("TRN2", target_bir_lowering=False, debug=False,
                   num_devices=NCORES)

    def din(name, shape, dt):
        return nc.dram_tensor(name, shape, dt, kind="ExternalInput").ap()

    d_x = {}
    for nm in ("Xh", "X1h"):
        d_x[nm] = din(nm, [H, CAP], xdt)
    if lowbits:
        for nm in ("Xl", "X1l"):
            d_x[nm] = din(nm, [H, CAP], xdt)
    d_w = {}
    for nm in ("W1h", "W2h", "W3h"):
        d_w[nm] = din(nm, [BR, H, H], wdt)
    if lowbits:
        for nm in ("W1l", "W2l", "W3l"):
            d_w[nm] = din(nm, [BR, H, H], wdt)
    d_bc12 = din("bc12", [BR, 128, MT], F32)
    d_bc3 = din("bc3", [BR, 128, MT], F32)
    d_w45h = din("W45h", [BR, H, 33], xdt)
    d_w45l = din("W45l", [BR, H, 33], xdt) if lowbits else None
    d_c12 = din("c12", [BR, 4, 2], F32)
    d_ones = din("onesd", [128, 1], xdt)
    d_y = nc.dram_tensor("y", [4, BR * CAP], F32, kind="ExternalOutput").ap()
    d_bounce = nc.dram_tensor("bounce", [BR * len(plan), 2 * NB], F32).ap()

    with TileContext(nc) as tc:
        wb = 2 if mode == "bf16" else 1
        xb = 2 if mode == "bf16" else 1
        with (
            tc.tile_pool(name="wpool", bufs=wb) as wpool,
            tc.tile_pool(name="xpool", bufs=xb) as xpool,
            tc.tile_pool(name="zpool", bufs=1) as zpool,
            tc.tile_pool(name="tpool", bufs=2) as tpool,
            tc.tile_pool(name="psum", bufs=4, space="PSUM") as pp,
            tc.tile_pool(name="psq", bufs=2, space="PSUM") as pps,
            tc.tile_pool(name="small", bufs=1) as sp,
            tc.tile_pool(name="persist", bufs=1) as pers,
        ):
            # persistent small tensors
            bc12 = pers.tile([128, BR * MT], F32, name="bc12s")
            bc3 = pers.tile([128, BR * MT], F32, name="bc3s")
            c12 = pers.tile([4, BR * 2], F32, name="c12s")
            for s in range(BR):
                nc.gpsimd.dma_start(out=bc12[:, s * MT:(s + 1) * MT], in_=d_bc12[s])
                nc.gpsimd.dma_start(out=bc3[:, s * MT:(s + 1) * MT], in_=d_bc3[s])
                nc.gpsimd.dma_start(out=c12[:, s * 2:(s + 1) * 2], in_=d_c12[s])
            w45h = [pers.tile([128, BR * 33], xdt, name=f"w45h{k}")
                    for k in range(KT)]
            w45l = ([pers.tile([128, BR * 33], xdt, name=f"w45l{k}")
                     for k in range(KT)] if lowbits else None)
            for k in range(KT):
                for s in range(BR):
                    nc.gpsimd.dma_start(out=w45h[k][:, s * 33:(s + 1) * 33],
                                        in_=d_w45h[s, k * 128:(k + 1) * 128, :])
                    if lowbits:
                        nc.gpsimd.dma_start(out=w45l[k][:, s * 33:(s + 1) * 33],
                                            in_=d_w45l[s, k * 128:(k + 1) * 128, :])
            ones1 = pers.tile([128, 1], xdt, name="ones1")
            nc.gpsimd.dma_start(out=ones1[:], in_=d_ones[:])
            epst = pers.tile([1, 1], F32, name="epst")
            nc.vector.memset(epst[:], EPS)
            y_sb = pers.tile([4, BR * CAP], F32, name="y_sb")

            env = {
                "plan": plan, "CAP": CAP,
                "xdt": xdt, "wdt": wdt, "d_x": d_x, "d_w": d_w,
                "d_bounce": d_bounce, "bc12": bc12, "bc3": bc3, "c12": c12,
                "w45h": w45h, "w45l": w45l, "ones1": ones1, "epst": epst,
                "y_sb": y_sb, "wpool": wpool, "xpool": xpool, "zpool": zpool,
                "tpool": tpool, "pp": pp, "pps": pps, "sp": sp,
            }
            for rep in range(reps):
                for s in range(BR):
                    _branch(nc, mode, s, env)

            nc.sync.dma_start(out=d_y[:], in_=y_sb[:])

    nc.compile()
    return nc


def _branch(nc, mode, s, env):
    """One branch (2 per core): z -> h -> LN-folded y, streamed over batch chunks."""
    import concourse.mybir as mybir
    from concourse.alu_op_type import AluOpType

    F32 = mybir.dt.float32
    AF = mybir.ActivationFunctionType
    lowbits = mode == "split3"
    xdt = env["xdt"]
    wdt = env["wdt"]
    wpool, xpool, zpool, tpool = (env["wpool"], env["xpool"], env["zpool"],
                                  env["tpool"])
    pp, pps, sp = env["pp"], env["pps"], env["sp"]
    d_x, d_w, d_bounce = env["d_x"], env["d_w"], env["d_bounce"]
    bc12, bc3, c12 = env["bc12"], env["bc3"], env["c12"]
    w45h, w45l, ones1, epst, y_sb = (env["w45h"], env["w45l"], env["ones1"],
                                     env["epst"], env["y_sb"])

    plan, CAP = env["plan"], env["CAP"]
    offs = [sum(plan[:i]) for i in range(len(plan))]

    def load_x(ch):
        w, off = plan[ch], offs[ch]
        xt = {}
        for nm in d_x:
            xt[nm] = [xpool.tile([128, w], xdt, name=f"{nm}c{k}", tag=f"{nm}c{k}")
                      for k in range(KT)]
            for k in range(KT):
                nc.sync.dma_start(
                    out=xt[nm][k][:],
                    in_=d_x[nm][k * 128:(k + 1) * 128, off:off + w])
        return xt

    # chunk-0 X is issued BEFORE the weights: the first z matmuls need it and
    # it is much smaller than the weight set. W3 loads last (h-phase only).
    xt0 = load_x(0)
    wt = {}
    wnames = (("W1h", "W2h", "W1l", "W2l", "W3h", "W3l")
              if lowbits else ("W1h", "W2h", "W3h"))
    for nm in wnames:
        wt[nm] = [wpool.tile([128, H], wdt, name=f"{nm}k{k}", tag=f"{nm}k{k}")
                  for k in range(KT)]
        for k in range(KT):
            nc.sync.dma_start(out=wt[nm][k][:],
                              in_=d_w[nm][s, k * 128:(k + 1) * 128, :])

    for ch in range(len(plan)):
        w, off = plan[ch], offs[ch]
        cb = s * len(plan) + ch
        xt = xt0 if ch == 0 else load_x(ch)

        if lowbits:
            zpasses = [("W1h", "Xh"), ("W1l", "Xh"), ("W1h", "Xl"),
                       ("W2h", "X1h"), ("W2l", "X1h"), ("W2h", "X1l")]
        else:
            zpasses = [("W1h", "Xh"), ("W2h", "X1h")]

        # ---- z = W1^T X + W2^T X1 (+bias) ----
        zh = []
        zl = []
        for m in range(MT):
            ps = pp.tile([128, w], F32, name="ps", tag="ps")
            npass = len(zpasses)
            for pi, (wn, xn) in enumerate(zpasses):
                for k in range(KT):
                    nc.tensor.matmul(
                        ps[:],
                        lhsT=wt[wn][k][:, m * 128:(m + 1) * 128],
                        rhs=xt[xn][k][:],
                        start=(pi == 0 and k == 0),
                        stop=(pi == npass - 1 and k == KT - 1))
            zh_m = zpool.tile([128, w], xdt, name=f"zh{m}", tag=f"zh{m}")
            bcol = bc12[:, s * MT + m:s * MT + m + 1]
            nc.scalar.activation(out=zh_m[:], in_=ps[:],
                                 func=AF.Identity, bias=bcol, scale=1.0)
            zh.append(zh_m)
            if lowbits:
                zl_m = zpool.tile([128, w], xdt, name=f"zl{m}", tag=f"zl{m}")
                nc.vector.scalar_tensor_tensor(
                    out=zl_m[:], in0=ps[:], scalar=bcol, in1=zh_m[:],
                    op0=AluOpType.add, op1=AluOpType.subtract)
                zl.append(zl_m)

        # ---- h = relu(W3^T z + b3) ----
        hh = []
        hl = []
        ps_q = pps.tile([1, w], F32, name="ps_q", tag="ps_q")
        for m in range(MT):
            ps = pp.tile([128, w], F32, name="ps", tag="ps")
            hsrc = ([("W3h", zh), ("W3l", zh), ("W3h", zl)]
                    if lowbits else [("W3h", zh)])
            npass = len(hsrc)
            for pi, (wn, zsrc) in enumerate(hsrc):
                for k in range(KT):
                    nc.tensor.matmul(
                        ps[:],
                        lhsT=wt[wn][k][:, m * 128:(m + 1) * 128],
                        rhs=zsrc[k][:],
                        start=(pi == 0 and k == 0),
                        stop=(pi == npass - 1 and k == KT - 1))
            bcol = bc3[:, s * MT + m:s * MT + m + 1]
            hh_m = zpool.tile([128, w], xdt, name=f"hh{m}", tag=f"hh{m}")
            if lowbits:
                hf_m = tpool.tile([128, w], F32, name="hf", tag="hf")
                nc.vector.tensor_scalar(out=hf_m[:], in0=ps[:],
                                        scalar1=bcol, scalar2=0.0,
                                        op0=AluOpType.add, op1=AluOpType.max)
                nc.scalar.activation(out=hh_m[:], in_=hf_m[:],
                                     func=AF.Identity, bias=0.0, scale=1.0)
                hl_m = zpool.tile([128, w], xdt, name=f"hl{m}", tag=f"hl{m}")
                nc.vector.tensor_tensor(hl_m[:], hf_m[:], hh_m[:],
                                        AluOpType.subtract)
                hl.append(hl_m)
            else:
                nc.scalar.activation(out=hh_m[:], in_=ps[:],
                                     func=AF.Relu, bias=bcol, scale=1.0)
            hh.append(hh_m)
            vv_m = tpool.tile([128, w], xdt, name="vv", tag="vv")
            nc.vector.tensor_mul(vv_m[:], hh_m[:], hh_m[:])
            nc.tensor.matmul(ps_q[:], lhsT=ones1[:], rhs=vv_m[:],
                             start=(m == 0), stop=(m == MT - 1))

        # ---- u = [W4g|1]^T h (ones at partition 32) ; q = 1^T (h*h) ----
        ps_u = pps.tile([33, w], F32, name="ps_u", tag="ps_u")
        usrc = ([(w45h, hh), (w45l, hh), (w45h, hl)]
                if lowbits else [(w45h, hh)])
        npass = len(usrc)
        for pi, (wsrc, hsrc) in enumerate(usrc):
            for k in range(KT):
                nc.tensor.matmul(
                    ps_u[:],
                    lhsT=wsrc[k][:, s * 33:(s + 1) * 33],
                    rhs=hsrc[k][:],
                    start=(pi == 0 and k == 0),
                    stop=(pi == npass - 1 and k == KT - 1))

        # ---- LN tail: y = rstd*u - (rstd*mu)*c1 + c2 ----
        mu = sp.tile([1, w], F32, name="mu", tag="mu")
        tt = sp.tile([1, w], F32, name="tt", tag="tt")
        row2 = sp.tile([1, 2 * w], F32, name="row2", tag="row2")
        nc.scalar.mul(mu[:], ps_u[32:33, :], 1.0 / H)
        nc.scalar.mul(tt[:], ps_q[:], 1.0 / H)
        nc.vector.tensor_mul(row2[:, 0:w], mu[:], mu[:])
        nc.vector.tensor_tensor(tt[:], tt[:], row2[:, 0:w], AluOpType.subtract)
        nc.scalar.activation(out=tt[:], in_=tt[:], func=AF.Sqrt,
                             bias=epst[0:1, :], scale=1.0)
        nc.vector.reciprocal(out=row2[:, 0:w], in_=tt[:])
        nc.vector.tensor_mul(row2[:, w:2 * w], mu[:], row2[:, 0:w])
        bc4 = sp.tile([4, 2 * w], F32, name="bc4", tag="bc4")
        nc.gpsimd.dma_start(out=d_bounce[cb:cb + 1, 0:2 * w], in_=row2[:])
        nc.gpsimd.dma_start(out=bc4[:],
                            in_=d_bounce[cb:cb + 1, 0:2 * w].partition_broadcast(4))
        t0 = sp.tile([4, w], F32, name="t0", tag="t0")
        nc.vector.tensor_mul(t0[:], ps_u[0:4, :], bc4[:, 0:w])
        nc.vector.scalar_tensor_tensor(
            out=t0[:], in0=bc4[:, w:2 * w],
            scalar=c12[:, s * 2:s * 2 + 1], in1=t0[:],
            op0=AluOpType.mult, op1=AluOpType.subtract)
        nc.scalar.activation(
            out=y_sb[:, s * CAP + off:s * CAP + off + w],
            in_=t0[:], func=AF.Identity,
            bias=c12[:, s * 2 + 1:s * 2 + 2], scale=-1.0)


def kernel(X, X1, W1, b1, W2, b2, W3, b3, gamma, beta, W4, b4, positions_y):
    from concourse.bass_utils import run_bass_kernel_spmd

    mode = MODE
    lowbits = mode == "split3"
    fp32in = mode in ("f32r", "f32")

    pos = np.asarray(positions_y).astype(np.int64)
    uniq, inverse = np.unique(pos, return_inverse=True)
    U = len(uniq)
    for plan in ((512, 512, 512), (512, 512, 512, 256), (512, 512, 512, 512)):
        if U <= sum(plan):
            break
    CAP = sum(plan)

    key = (mode, plan)
    if key not in _cache:
        _cache[key] = _build(mode, plan=plan)
    nc = _cache[key]
    global _last_nc, _last_plan
    _last_nc, _last_plan = nc, plan

    def gather_pad(M):
        t = np.zeros((H, CAP), np.float32)
        t[:, :U] = np.asarray(M, np.float32)[uniq].T
        return np.ascontiguousarray(t)

    Xt = gather_pad(X)                                              # [H, CAP]
    X1t = gather_pad(X1)

    def cvt(a):
        return np.ascontiguousarray(a, np.float32) if fp32in else _hi(a)

    in_maps = []
    for c in range(NCORES):
        m = {"Xh": cvt(Xt), "X1h": cvt(X1t)}
        if lowbits:
            m["Xl"] = _lo(Xt)
            m["X1l"] = _lo(X1t)
        for base, Wt in (("W1", W1), ("W2", W2), ("W3", W3)):
            sl = np.asarray(Wt, np.float32)[2 * c:2 * c + 2]        # [BR, H, H]
            m[base + "h"] = cvt(sl)
            if lowbits:
                m[base + "l"] = _lo(sl)
        b12 = np.asarray(b1, np.float32)[2 * c:2 * c + 2] + \
            np.asarray(b2, np.float32)[2 * c:2 * c + 2]             # [BR, H]
        m["bc12"] = np.ascontiguousarray(
            b12.reshape(BR, MT, 128).transpose(0, 2, 1))            # [BR,128,MT]
        b3s = np.asarray(b3, np.float32)[2 * c:2 * c + 2]
        m["bc3"] = np.ascontiguousarray(
            b3s.reshape(BR, MT, 128).transpose(0, 2, 1))
        w45h = np.zeros((BR, H, 33), np.float32)
        c12a = np.zeros((BR, 4, 2), np.float32)
        for s in range(BR):
            a = 2 * c + s
            W4g = (np.asarray(W4, np.float32)[a]
                   * np.asarray(gamma, np.float32)[a][:, None])     # [H, 4]
            w45h[s, :, 0:4] = W4g
            w45h[s, :, 32] = 1.0
            c12a[s, :, 0] = W4g.sum(0)
            c12a[s, :, 1] = (np.asarray(W4, np.float32)[a].T
                             @ np.asarray(beta, np.float32)[a]) \
                + np.asarray(b4, np.float32)[a]
        if fp32in:
            m["W45h"] = np.ascontiguousarray(w45h)
        else:
            m["W45h"] = _hi(w45h)
        if lowbits:
            wl = _lo(w45h)
            wl[:, :, 32] = 0.0   # ones column must not double-count
            m["W45l"] = wl
        m["c12"] = np.ascontiguousarray(c12a)
        m["onesd"] = (np.ones((128, 1), np.float32) if fp32in
                      else np.ones((128, 1), bf16))
        in_maps.append(m)

    global _last_in_maps
    _last_in_maps = in_maps
    res = run_bass_kernel_spmd(nc, in_maps, list(range(NCORES)))

    out = np.empty((P, A * O), np.float32)
    for c in range(NCORES):
        yc = res.results[c]["y"]                                    # [4, BR*CAP]
        for s in range(BR):
            a = 2 * c + s
            out[:, a * O:(a + 1) * O] = yc[:, s * CAP + inverse].T
    return out


# revision 21
# speedup vs baseline: 1.0572x; 1.0163x over previous
"""Trainium2 Bass kernel for nn_AbPred_both (dense per-branch MLP + LayerNorm + row gather).

Strategy:
  - Only the P=2048 rows selected by positions_y are computed (the reference's
    computation is row-wise, so Y[positions_y] == f(X[positions_y], X1[positions_y])).
  - Branch (antibody) axis A=16 is sharded across 8 cores, 2 branches per core.
  - All activations live transposed on-chip ([H, batch]) so every matmul
    contracts over the partition axis, with the stored weight layout [H_in, H_out]
    serving directly as lhsT.
  - Matmuls run in float32r (full PE rate, ~tf32 accuracy; measured 1.5e-4
    per-layer rel err on HW vs 2.4e-3 for bf16).
  - LayerNorm is folded into the final tiny [H,4] matmul:
        y^T = rstd * (W4g^T h^T) - (rstd*mu) * c1 + c2
    with W4g = W4*gamma, c1 = sum_h W4g, c2 = W4^T beta + b4 (host precomputed),
    and mu/rstd from column sums of h and h*h obtained via a ones column packed
    into the W4g matmul (at partition 32 so PSUM reads stay 32-aligned).
"""
import numpy as np
import ml_dtypes

bf16 = ml_dtypes.bfloat16

H = 1024
A = 16
O = 4
B = 4096
P = 2048
EPS = 1e-5
NCORES = 8
BR = A // NCORES     # branches per core
NB = 512             # batch chunk (columns per matmul)
NCH = P // NB        # chunks
KT = H // 128
MT = H // 128

MODE = "f32r"        # one of: bf16, split3, f32r, f32

_cache = {}
_last_in_maps = None
_last_nc = None
_last_plan = None


def _hi(a):
    return np.ascontiguousarray(a.astype(bf16))


def _lo(a):
    return np.ascontiguousarray(
        (a.astype(np.float64) - a.astype(bf16).astype(np.float64))
        .astype(np.float32).astype(bf16))


def _build(mode, reps=1, plan=(512, 512, 512, 512)):
    CAP = sum(plan)
    import concourse.mybir as mybir
    from concourse import bacc
    from concourse.alu_op_type import AluOpType
    from concourse.tile import TileContext

    F32 = mybir.dt.float32
    F32R = mybir.dt.float32r
    BF = mybir.dt.bfloat16
    AF = mybir.ActivationFunctionType

    lowbits = mode == "split3"
    if mode == "f32r":
        xdt = wdt = F32R
    elif mode == "f32":
        xdt = wdt = F32
    else:
        xdt = wdt = BF

    nc = bacc.Bacc("TRN2", target_bir_lowering=False, debug=False,
                   num_devices=NCORES)

    def din(name, shape, dt):
        return nc.dram_tensor(name, shape, dt, kind="ExternalInput").ap()

    d_x = {}
    for nm in ("Xh", "X1h"):
        d_x[nm] = din(nm, [H, CAP], xdt)
    if lowbits:
        for nm in ("Xl", "X1l"):
            d_x[nm] = din(nm, [H, CAP], xdt)
    d_w = {}
    for nm in ("W1h", "W2h", "W3h"):
        d_w[nm] = din(nm, [BR, H, H], wdt)
    if lowbits:
        for nm in ("W1l", "W2l", "W3l"):
            d_w[nm] = din(nm, [BR, H, H], wdt)
    d_bc12 = din("bc12", [BR, 128, MT], F32)
    d_bc3 = din("bc3", [BR, 128, MT], F32)
    d_w45h = din("W45h", [BR, H, 33], xdt)
    d_w45l = din("W45l", [BR, H, 33], xdt) if lowbits else None
    d_c12 = din("c12", [BR, 4, 2], F32)
    d_ones = din("onesd", [128, 1], xdt)
    d_y = nc.dram_tensor("y", [4, BR * CAP], F32, kind="ExternalOutput").ap()
    d_bounce = nc.dram_tensor("bounce", [BR * len(plan), 2 * NB], F32).ap()

    with TileContext(nc) as tc:
        wb = 2 if mode == "bf16" else 1
        xb = 2 if mode == "bf16" else 1
        with (
            tc.tile_pool(name="wpool", bufs=wb) as wpool,
            tc.tile_pool(name="xpool", bufs=xb) as xpool,
            tc.tile_pool(name="zpool", bufs=1) as zpool,
            tc.tile_pool(name="tpool", bufs=2) as tpool,
            tc.tile_pool(name="psum", bufs=6, space="PSUM") as pp,
            tc.tile_pool(name="psq", bufs=1, space="PSUM") as pps,
            tc.tile_pool(name="small", bufs=1) as sp,
            tc.tile_pool(name="persist", bufs=1) as pers,
        ):
            # persistent small tensors
            bc12 = pers.tile([128, BR * MT], F32, name="bc12s")
            bc3 = pers.tile([128, BR * MT], F32, name="bc3s")
            c12 = pers.tile([4, BR * 2], F32, name="c12s")
            for s in range(BR):
                nc.gpsimd.dma_start(out=bc12[:, s * MT:(s + 1) * MT], in_=d_bc12[s])
                nc.gpsimd.dma_start(out=bc3[:, s * MT:(s + 1) * MT], in_=d_bc3[s])
                nc.gpsimd.dma_start(out=c12[:, s * 2:(s + 1) * 2], in_=d_c12[s])
            w45h = [pers.tile([128, BR * 33], xdt, name=f"w45h{k}")
                    for k in range(KT)]
            w45l = ([pers.tile([128, BR * 33], xdt, name=f"w45l{k}")
                     for k in range(KT)] if lowbits else None)
            for k in range(KT):
                for s in range(BR):
                    nc.gpsimd.dma_start(out=w45h[k][:, s * 33:(s + 1) * 33],
                                        in_=d_w45h[s, k * 128:(k + 1) * 128, :])
                    if lowbits:
                        nc.gpsimd.dma_start(out=w45l[k][:, s * 33:(s + 1) * 33],
                                            in_=d_w45l[s, k * 128:(k + 1) * 128, :])
            ones1 = pers.tile([128, 1], xdt, name="ones1")
            nc.gpsimd.dma_start(out=ones1[:], in_=d_ones[:])
            epst = pers.tile([1, 1], F32, name="epst")
            nc.vector.memset(epst[:], EPS)
            y_sb = pers.tile([4, BR * CAP], F32, name="y_sb")

            env = {
                "plan": plan, "CAP": CAP,
                "xdt": xdt, "wdt": wdt, "d_x": d_x, "d_w": d_w,
                "d_bounce": d_bounce, "bc12": bc12, "bc3": bc3, "c12": c12,
                "w45h": w45h, "w45l": w45l, "ones1": ones1, "epst": epst,
                "y_sb": y_sb, "wpool": wpool, "xpool": xpool, "zpool": zpool,
                "tpool": tpool, "pp": pp, "pps": pps, "sp": sp,
            }
            env["d_y"] = d_y
            for rep in range(reps):
                for s in range(BR):
                    _branch(nc, mode, s, env)

    nc.compile()
    return nc


def _branch(nc, mode, s, env):
    """One branch (2 per core): z -> h -> LN-folded y, streamed over batch chunks."""
    import concourse.mybir as mybir
    from concourse.alu_op_type import AluOpType

    F32 = mybir.dt.float32
    AF = mybir.ActivationFunctionType
    lowbits = mode == "split3"
    xdt = env["xdt"]
    wdt = env["wdt"]
    wpool, xpool, zpool, tpool = (env["wpool"], env["xpool"], env["zpool"],
                                  env["tpool"])
    pp, pps, sp = env["pp"], env["pps"], env["sp"]
    d_x, d_w, d_bounce = env["d_x"], env["d_w"], env["d_bounce"]
    bc12, bc3, c12 = env["bc12"], env["bc3"], env["c12"]
    w45h, w45l, ones1, epst, y_sb = (env["w45h"], env["w45l"], env["ones1"],
                                     env["epst"], env["y_sb"])

    plan, CAP = env["plan"], env["CAP"]
    offs = [sum(plan[:i]) for i in range(len(plan))]

    def load_x(ch):
        w, off = plan[ch], offs[ch]
        xt = {}
        for nm in d_x:
            xt[nm] = [xpool.tile([128, w], xdt, name=f"{nm}c{k}", tag=f"{nm}c{k}")
                      for k in range(KT)]
            for k in range(KT):
                nc.sync.dma_start(
                    out=xt[nm][k][:],
                    in_=d_x[nm][k * 128:(k + 1) * 128, off:off + w])
        return xt

    # chunk-0 X is issued BEFORE the weights: the first z matmuls need it and
    # it is much smaller than the weight set. W3 loads last (h-phase only).
    xt0 = load_x(0)
    wt = {}
    wnames = (("W1h", "W2h", "W1l", "W2l", "W3h", "W3l")
              if lowbits else ("W1h", "W2h", "W3h"))
    for nm in wnames:
        wt[nm] = [wpool.tile([128, H], wdt, name=f"{nm}k{k}", tag=f"{nm}k{k}")
                  for k in range(KT)]
        for k in range(KT):
            nc.sync.dma_start(out=wt[nm][k][:],
                              in_=d_w[nm][s, k * 128:(k + 1) * 128, :])

    for ch in range(len(plan)):
        w, off = plan[ch], offs[ch]
        cb = s * len(plan) + ch
        xt = xt0 if ch == 0 else load_x(ch)

        if lowbits:
            zpasses = [("W1h", "Xh"), ("W1l", "Xh"), ("W1h", "Xl"),
                       ("W2h", "X1h"), ("W2l", "X1h"), ("W2h", "X1l")]
        else:
            zpasses = [("W1h", "Xh"), ("W2h", "X1h")]

        # ---- z = W1^T X + W2^T X1 (+bias) ----
        zh = []
        zl = []
        for m in range(MT):
            ps = pp.tile([128, w], F32, name="ps", tag="ps")
            npass = len(zpasses)
            for pi, (wn, xn) in enumerate(zpasses):
                for k in range(KT):
                    nc.tensor.matmul(
                        ps[:],
                        lhsT=wt[wn][k][:, m * 128:(m + 1) * 128],
                        rhs=xt[xn][k][:],
                        start=(pi == 0 and k == 0),
                        stop=(pi == npass - 1 and k == KT - 1))
            zh_m = zpool.tile([128, w], xdt, name=f"zh{m}", tag=f"zh{m}")
            bcol = bc12[:, s * MT + m:s * MT + m + 1]
            nc.scalar.activation(out=zh_m[:], in_=ps[:],
                                 func=AF.Identity, bias=bcol, scale=1.0)
            zh.append(zh_m)
            if lowbits:
                zl_m = zpool.tile([128, w], xdt, name=f"zl{m}", tag=f"zl{m}")
                nc.vector.scalar_tensor_tensor(
                    out=zl_m[:], in0=ps[:], scalar=bcol, in1=zh_m[:],
                    op0=AluOpType.add, op1=AluOpType.subtract)
                zl.append(zl_m)

        # ---- h = relu(W3^T z + b3) ----
        hh = []
        hl = []
        ps_q = pps.tile([1, w], F32, name="ps_q", tag="ps_q")
        for m in range(MT):
            ps = pp.tile([128, w], F32, name="ps", tag="ps")
            hsrc = ([("W3h", zh), ("W3l", zh), ("W3h", zl)]
                    if lowbits else [("W3h", zh)])
            npass = len(hsrc)
            for pi, (wn, zsrc) in enumerate(hsrc):
                for k in range(KT):
                    nc.tensor.matmul(
                        ps[:],
                        lhsT=wt[wn][k][:, m * 128:(m + 1) * 128],
                        rhs=zsrc[k][:],
                        start=(pi == 0 and k == 0),
                        stop=(pi == npass - 1 and k == KT - 1))
            bcol = bc3[:, s * MT + m:s * MT + m + 1]
            hh_m = zpool.tile([128, w], xdt, name=f"hh{m}", tag=f"hh{m}")
            if lowbits:
                hf_m = tpool.tile([128, w], F32, name="hf", tag="hf")
                nc.vector.tensor_scalar(out=hf_m[:], in0=ps[:],
                                        scalar1=bcol, scalar2=0.0,
                                        op0=AluOpType.add, op1=AluOpType.max)
                nc.scalar.activation(out=hh_m[:], in_=hf_m[:],
                                     func=AF.Identity, bias=0.0, scale=1.0)
                hl_m = zpool.tile([128, w], xdt, name=f"hl{m}", tag=f"hl{m}")
                nc.vector.tensor_tensor(hl_m[:], hf_m[:], hh_m[:],
                                        AluOpType.subtract)
                hl.append(hl_m)
            else:
                nc.scalar.activation(out=hh_m[:], in_=ps[:],
                                     func=AF.Relu, bias=bcol, scale=1.0)
            hh.append(hh_m)
            vv_m = tpool.tile([128, w], xdt, name="vv", tag="vv")
            nc.vector.tensor_mul(vv_m[:], hh_m[:], hh_m[:])
            nc.tensor.matmul(ps_q[:], lhsT=ones1[:], rhs=vv_m[:],
                             start=(m == 0), stop=(m == MT - 1))

        # ---- u = [W4g|1]^T h (ones at partition 32) ; q = 1^T (h*h) ----
        ps_u = pps.tile([33, w], F32, name="ps_u", tag="ps_u")
        usrc = ([(w45h, hh), (w45l, hh), (w45h, hl)]
                if lowbits else [(w45h, hh)])
        npass = len(usrc)
        for pi, (wsrc, hsrc) in enumerate(usrc):
            for k in range(KT):
                nc.tensor.matmul(
                    ps_u[:],
                    lhsT=wsrc[k][:, s * 33:(s + 1) * 33],
                    rhs=hsrc[k][:],
                    start=(pi == 0 and k == 0),
                    stop=(pi == npass - 1 and k == KT - 1))

        # ---- LN tail: y = rstd*u - (rstd*mu)*c1 + c2 ----
        mu = sp.tile([1, w], F32, name="mu", tag="mu")
        tt = sp.tile([1, w], F32, name="tt", tag="tt")
        row2 = sp.tile([1, 2 * w], F32, name="row2", tag="row2")
        nc.scalar.mul(mu[:], ps_u[32:33, :], 1.0 / H)
        nc.scalar.mul(tt[:], ps_q[:], 1.0 / H)
        nc.vector.tensor_mul(row2[:, 0:w], mu[:], mu[:])
        nc.vector.tensor_tensor(tt[:], tt[:], row2[:, 0:w], AluOpType.subtract)
        nc.scalar.activation(out=tt[:], in_=tt[:], func=AF.Sqrt,
                             bias=epst[0:1, :], scale=1.0)
        nc.vector.reciprocal(out=row2[:, 0:w], in_=tt[:])
        nc.vector.tensor_mul(row2[:, w:2 * w], mu[:], row2[:, 0:w])
        bc4 = sp.tile([4, 2 * w], F32, name="bc4", tag="bc4")
        nc.gpsimd.dma_start(out=d_bounce[cb:cb + 1, 0:2 * w], in_=row2[:])
        nc.gpsimd.dma_start(out=bc4[:],
                            in_=d_bounce[cb:cb + 1, 0:2 * w].partition_broadcast(4))
        t0 = sp.tile([4, w], F32, name="t0", tag="t0")
        nc.vector.tensor_mul(t0[:], ps_u[0:4, :], bc4[:, 0:w])
        nc.vector.scalar_tensor_tensor(
            out=t0[:], in0=bc4[:, w:2 * w],
            scalar=c12[:, s * 2:s * 2 + 1], in1=t0[:],
            op0=AluOpType.mult, op1=AluOpType.subtract)
        nc.scalar.activation(
            out=y_sb[:, s * CAP + off:s * CAP + off + w],
            in_=t0[:], func=AF.Identity,
            bias=c12[:, s * 2 + 1:s * 2 + 2], scale=-1.0)

    nc.sync.dma_start(out=env["d_y"][:, s * CAP:(s + 1) * CAP],
                      in_=y_sb[:, s * CAP:(s + 1) * CAP])


def kernel(X, X1, W1, b1, W2, b2, W3, b3, gamma, beta, W4, b4, positions_y):
    from concourse.bass_utils import run_bass_kernel_spmd

    mode = MODE
    lowbits = mode == "split3"
    fp32in = mode in ("f32r", "f32")

    pos = np.asarray(positions_y).astype(np.int64)
    uniq, inverse = np.unique(pos, return_inverse=True)
    U = len(uniq)
    for plan in ((512, 512, 512), (512, 512, 512, 256), (512, 512, 512, 512)):
        if U <= sum(plan):
            break
    CAP = sum(plan)

    key = (mode, plan)
    if key not in _cache:
        _cache[key] = _build(mode, plan=plan)
    nc = _cache[key]
    global _last_nc, _last_plan
    _last_nc, _last_plan = nc, plan

    def gather_pad(M):
        t = np.zeros((H, CAP), np.float32)
        t[:, :U] = np.asarray(M, np.float32)[uniq].T
        return np.ascontiguousarray(t)

    Xt = gather_pad(X)                                              # [H, CAP]
    X1t = gather_pad(X1)

    def cvt(a):
        return np.ascontiguousarray(a, np.float32) if fp32in else _hi(a)

    in_maps = []
    for c in range(NCORES):
        m = {"Xh": cvt(Xt), "X1h": cvt(X1t)}
        if lowbits:
            m["Xl"] = _lo(Xt)
            m["X1l"] = _lo(X1t)
        for base, Wt in (("W1", W1), ("W2", W2), ("W3", W3)):
            sl = np.asarray(Wt, np.float32)[2 * c:2 * c + 2]        # [BR, H, H]
            m[base + "h"] = cvt(sl)
            if lowbits:
                m[base + "l"] = _lo(sl)
        b12 = np.asarray(b1, np.float32)[2 * c:2 * c + 2] + \
            np.asarray(b2, np.float32)[2 * c:2 * c + 2]             # [BR, H]
        m["bc12"] = np.ascontiguousarray(
            b12.reshape(BR, MT, 128).transpose(0, 2, 1))            # [BR,128,MT]
        b3s = np.asarray(b3, np.float32)[2 * c:2 * c + 2]
        m["bc3"] = np.ascontiguousarray(
            b3s.reshape(BR, MT, 128).transpose(0, 2, 1))
        w45h = np.zeros((BR, H, 33), np.float32)
        c12a = np.zeros((BR, 4, 2), np.float32)
        for s in range(BR):
            a = 2 * c + s
            W4g = (np.asarray(W4, np.float32)[a]
                   * np.asarray(gamma, np.float32)[a][:, None])     # [H, 4]
            w45h[s, :, 0:4] = W4g
            w45h[s, :, 32] = 1.0
            c12a[s, :, 0] = W4g.sum(0)
            c12a[s, :, 1] = (np.asarray(W4, np.float32)[a].T
                             @ np.asarray(beta, np.float32)[a]) \
                + np.asarray(b4, np.float32)[a]
        if fp32in:
            m["W45h"] = np.ascontiguousarray(w45h)
        else:
            m["W45h"] = _hi(w45h)
        if lowbits:
            wl = _lo(w45h)
            wl[:, :, 32] = 0.0   # ones column must not double-count
            m["W45l"] = wl
        m["c12"] = np.ascontiguousarray(c12a)
        m["onesd"] = (np.ones((128, 1), np.float32) if fp32in
                      else np.ones((128, 1), bf16))
        in_maps.append(m)

    global _last_in_maps
    _last_in_maps = in_maps
    res = run_bass_kernel_spmd(nc, in_maps, list(range(NCORES)))

    out = np.empty((P, A * O), np.float32)
    for c in range(NCORES):
        yc = res.results[c]["y"]                                    # [4, BR*CAP]
        for s in range(BR):
            a = 2 * c + s
            out[:, a * O:(a + 1) * O] = yc[:, s * CAP + inverse].T
    return out
